# revision 15
# baseline (speedup 1.0000x reference)
"""Causal self-attention (B=4, T=2048, C=1024, H=16, D=64) on 8 TRN2 NeuronCores.

Sharding: core c handles batch b = c//2 and head-group hg = c%2 (8 heads each).
Each core computes, for its (b, hg):
  - Q^T, K^T ([512, 2048], head-major transposed) and V ([2048, 512], natural)
    from x[b]^T via f32r matmuls (weights column-sliced per head group)
  - causal attention per head in "S^T layout": S^T[k, q] chunks via bf16
    matmuls, exp on ScalarE (PSUM->SBUF, scale=1/8, no max-subtraction --
    scores are O(few) so exp is safe in f32), causal masking via a
    precomputed step-mask multiply on the diagonal chunks only, and
    PV via matmul with a ones-column appended to V (output row 64 = softmax
    denominator), accumulated in PSUM over k-chunks
  - normalize via reciprocal + partition-broadcast + multiply -> y^T
  - output projection y^T @ W_proj[hg rows] -> partial out [2048, 1024]
Host sums the two head-group partials per batch and adds b_proj.
"""

import os

import numpy as np
import ml_dtypes

import concourse.bass as bass
import concourse.tile as tile
from concourse import bacc, mybir
from concourse.bass_utils import run_bass_kernel_spmd

F32 = mybir.dt.float32
F32R = mybir.dt.float32r
BF16 = mybir.dt.bfloat16
# dtype for the projection (QKV / out-proj) matmul operands: bf16 is 2x faster
# on the PE xbus than f32r (2B/cycle/partition stream width)
MM_DT = BF16 if os.environ.get("KMMDT", "bf16") == "bf16" else F32R
NP_MM_DT = ml_dtypes.bfloat16 if os.environ.get("KMMDT", "bf16") == "bf16" else np.float32
EXP = mybir.ActivationFunctionType.Exp

B, T, C = 4, 2048, 1024
H, D = 16, 64
HPC = 8          # heads per core
CC = C // 128    # 8 contraction chunks
G = T // 512     # 4 q-blocks of 512
NC_CORES = 8

LAST_EXEC_NS = None

_NC_CACHE = None





def _build_nc():
    stage = os.environ.get("KSTAGE", "all")  # debug gate: qkv | attn | nobcast | all
    nc = bacc.Bacc("TRN2", target_bir_lowering=False, debug=False)

    xT_d = nc.dram_tensor("xT", [C, T], MM_DT, kind="ExternalInput")
    wqkv_d = nc.dram_tensor("wqkv", [C, 1536], MM_DT, kind="ExternalInput")
    wproj_d = nc.dram_tensor("wproj", [512, C], MM_DT, kind="ExternalInput")
    bqk_d = nc.dram_tensor("bqk", [128, 8], F32, kind="ExternalInput")
    wvb_d = nc.dram_tensor("wvb", [1, 512], MM_DT, kind="ExternalInput")
    onesr_d = nc.dram_tensor("onesr", [1, 128], MM_DT, kind="ExternalInput")
    m0_d = nc.dram_tensor("m0x2", [128, 256], BF16, kind="ExternalInput")
    out_d = nc.dram_tensor("out", [T, C], F32, kind="ExternalOutput")

    with tile.TileContext(nc) as tc:
        with (
            tc.tile_pool(name="consts", bufs=1) as cp,
            tc.tile_pool(name="kv", bufs=1) as kvp,
            tc.tile_pool(name="stream", bufs=1) as sp,
            tc.tile_pool(name="ps", bufs=1, space="PSUM") as ps,
        ):
            # ---- constants / weights ----
            wqkv = []
            for c in range(CC):
                w = sp.tile([128, 1536], MM_DT, tag=f"wq{c}", name=f"wqkv{c}")
                nc.sync.dma_start(w[:], wqkv_d[c * 128:(c + 1) * 128, :])
                wqkv.append(w)
            wproj = []
            for dch in range(4):
                w = cp.tile([128, C], MM_DT, tag=f"wp{dch}", name=f"wproj{dch}")
                nc.sync.dma_start(w[:], wproj_d[dch * 128:(dch + 1) * 128, :])
                wproj.append(w)
            bqk = cp.tile([128, 8], F32, tag="bqk")
            nc.sync.dma_start(bqk[:], bqk_d[:])
            wvb = cp.tile([1, 512], MM_DT, tag="wvb")
            nc.sync.dma_start(wvb[:], wvb_d[:])
            m0 = cp.tile([128, 256], BF16, tag="m0")
            nc.sync.dma_start(m0[:], m0_d[:])
            ones = cp.tile([1, 128], MM_DT, tag="ones")
            nc.sync.dma_start(ones[:], onesr_d[:])

            # ---- persistent K^T and V(+ones) caches ----
            kt = []
            for hp in range(4):
                t = kvp.tile([128, T], BF16, tag=f"kt{hp}", name=f"kt{hp}")
                kt.append(t)
            vo = []
            for t16 in range(16):
                v = kvp.tile([128, HPC * 65], BF16, tag=f"vo{t16}", name=f"vo{t16}")
                nc.gpsimd.memset(v[:], 1.0)
                vo.append(v)

            for g in range(G):
                # ================= QKV projection for t-block g =================
                xts = []
                for c in range(CC):
                    xt = sp.tile([128, 512], MM_DT, tag="xt", bufs=10, name=f"xt{g}_{c}")
                    nc.sync.dma_start(
                        xt[:], xT_d[c * 128:(c + 1) * 128, g * 512:(g + 1) * 512]
                    )
                    xts.append(xt)

                # Q^T for this q-block: 4 partition-tiles (2 heads each)
                qts = []
                for hp in range(4):
                    p = ps.tile([128, 512], F32, tag="qk", bufs=2, name=f"psq{g}_{hp}")
                    for c in range(CC):
                        nc.tensor.matmul(
                            p[:],
                            wqkv[c][:, hp * 128:(hp + 1) * 128],
                            xts[c][:],
                            start=(c == 0),
                            stop=(c == CC - 1),
                        )
                    q = sp.tile([128, 512], BF16, tag=f"qt{hp}", bufs=2,
                                name=f"qt{g}_{hp}")
                    nc.vector.tensor_scalar_add(q[:], p[:], bqk[:, hp:hp + 1])
                    qts.append(q)

                # K^T block g -> persistent cache
                for hp in range(4):
                    p = ps.tile([128, 512], F32, tag="qk", bufs=2, name=f"psk{g}_{hp}")
                    for c in range(CC):
                        nc.tensor.matmul(
                            p[:],
                            wqkv[c][:, 512 + hp * 128:512 + (hp + 1) * 128],
                            xts[c][:],
                            start=(c == 0),
                            stop=(c == CC - 1),
                        )
                    nc.vector.tensor_scalar_add(
                        kt[hp][:, g * 512:(g + 1) * 512], p[:], bqk[:, 4 + hp:5 + hp]
                    )

                # V natural for 4 t-chunks of block g -> Vones cache
                for t4 in range(4):
                    t16 = 4 * g + t4
                    p = ps.tile([128, 512], F32, tag="qk", bufs=2, name=f"psv{t16}")
                    for c in range(CC):
                        nc.tensor.matmul(
                            p[:],
                            xts[c][:, t4 * 128:(t4 + 1) * 128],
                            wqkv[c][:, 1024:1536],
                            start=(c == 0),
                            stop=False,
                        )
                    nc.tensor.matmul(
                        p[:], ones[:], wvb[:], start=False, stop=True
                    )
                    vslice = vo[t16][:].rearrange("p (h c) -> p h c", h=HPC)[:, :, 0:64]
                    psrc = p[:].rearrange("p (h c) -> p h c", h=HPC)
                    nc.vector.tensor_copy(vslice, psrc)

                # ================= attention for q-block g =================
                if stage == "qkv":
                    continue
                # chunk descriptors: (kc, q-window offset, width, needs_mask)
                chunks = [(kc, 0, 512, False) for kc in range(4 * g)]
                chunks += [(4 * g + j, 128 * j, 512 - 128 * j, True)
                           for j in range(4)]
                nch = len(chunks)
                for hp in range(4):
                    pvs = []
                    for hs in range(2):
                        pv = ps.tile([65, 512], F32, tag="pv", bufs=2,
                                     name=f"pv{g}_{hp}_{hs}")
                        pvs.append(pv)
                    es = {}

                    def emit_pv(idx):
                        kc, qoff, w, _ = chunks[idx]
                        for hs in range(2):
                            h = 2 * hp + hs
                            nc.tensor.matmul(
                                pvs[hs][:, qoff:qoff + w],
                                vo[kc][:, h * 65:(h + 1) * 65],
                                es[(idx, hs)][:, 0:w],
                                start=(kc == 0), stop=(kc == 4 * g + 3),
                            )

                    for idx, (kc, qoff, w, masked) in enumerate(chunks):
                        sgs = []
                        for hs in range(2):
                            r0 = 64 * hs
                            sg = ps.tile([128, 512], F32, tag="sg", bufs=4,
                                         name=f"s{g}_{hp}_{hs}_{kc}")
                            nc.tensor.matmul(
                                sg[:, 0:w],
                                kt[hp][r0:r0 + 64, kc * 128:(kc + 1) * 128],
                                qts[hp][r0:r0 + 64, qoff:qoff + w],
                                start=True, stop=True,
                            )
                            sgs.append(sg)
                        for hs in range(2):
                            e = sp.tile([128, 512], BF16, tag="e", bufs=8,
                                        name=f"e{g}_{hp}_{hs}_{kc}")
                            nc.scalar.activation(e[:, 0:w], sgs[hs][:, 0:w],
                                                 EXP, scale=0.125)
                            if masked:
                                nc.vector.tensor_mul(
                                    e[:, 0:128], e[:, 0:128], m0[:, 0:128]
                                )
                            es[(idx, hs)] = e
                        if idx >= 2:
                            emit_pv(idx - 2)
                    for idx in range(max(0, nch - 2), nch):
                        emit_pv(idx)

                    # normalize both heads of the pair -> y tile rows
                    if hp == 0:
                        yts = []
                    yt = sp.tile([128, 512], MM_DT, tag=f"yt{hp}", bufs=2,
                                 name=f"yt{g}_{hp}")
                    for hs in range(2):
                        pv = pvs[hs]
                        if stage in ("attn", "nobcast"):
                            nc.vector.tensor_copy(
                                yt[64 * hs:64 * hs + 64, :], pv[0:64, :]
                            )
                            continue
                        # evacuate the PV accumulator promptly (frees the bank)
                        yU = sp.tile([64, 512], F32, tag="yU", bufs=3,
                                     name=f"yU{g}_{hp}_{hs}")
                        nc.vector.tensor_copy(yU[:], pv[0:64, :])
                        sc = sp.tile([1, 512], F32, tag="sc", bufs=2,
                                     name=f"sc{g}_{hp}_{hs}")
                        nc.scalar.copy(sc[:], pv[64:65, :])
                        r = sp.tile([1, 512], F32, tag="r", bufs=2,
                                    name=f"r{g}_{hp}_{hs}")
                        nc.vector.reciprocal_approx_fast(out=r[:], in_=sc[:])
                        rb = sp.tile([64, 512], F32, tag="rb", bufs=2,
                                     name=f"rb{g}_{hp}_{hs}")
                        nc.gpsimd.partition_broadcast(rb[:], r[:])
                        nc.vector.tensor_mul(
                            yt[64 * hs:64 * hs + 64, :], yU[:], rb[:]
                        )
                    yts.append(yt)

                # ================= output projection for t-block g =================
                if stage == "attn":
                    continue
                for t4 in range(4):
                    for c2 in range(2):
                        p = ps.tile([128, 512], F32, tag="qk", bufs=2,
                                    name=f"pso{g}_{t4}_{c2}")
                        for dch in range(4):
                            nc.tensor.matmul(
                                p[:],
                                yts[dch][:, t4 * 128:(t4 + 1) * 128],
                                wproj[dch][:, c2 * 512:(c2 + 1) * 512],
                                start=(dch == 0),
                                stop=(dch == 3),
                            )
                        o = sp.tile([128, 512], F32, tag="o", bufs=4,
                                    name=f"o{g}_{t4}_{c2}")
                        nc.vector.tensor_copy(o[:], p[:])
                        nc.sync.dma_start(
                            out_d[g * 512 + t4 * 128: g * 512 + (t4 + 1) * 128,
                                  c2 * 512:(c2 + 1) * 512],
                            o[:],
                        )

    nc.compile()
    return nc


def _get_nc():
    global _NC_CACHE
    if _NC_CACHE is None:
        _NC_CACHE = _build_nc()
    return _NC_CACHE


def _shard_inputs(x, W_qkv, b_qkv, W_proj):
    """Build the per-core input maps."""
    k0 = np.arange(128)
    m0 = (k0[:, None] <= np.arange(128)[None, :]).astype(ml_dtypes.bfloat16)
    m0x2 = np.concatenate([m0, m0], axis=1)

    in_maps = []
    for core in range(NC_CORES):
        b, hg = core // 2, core % 2
        sl = slice(hg * 512, (hg + 1) * 512)
        wq = W_qkv[:, 0 * C:1 * C][:, sl]
        wk = W_qkv[:, 1 * C:2 * C][:, sl]
        wv = W_qkv[:, 2 * C:3 * C][:, sl]
        bq = b_qkv[0 * C:1 * C][sl]
        bk = b_qkv[1 * C:2 * C][sl]
        bv = b_qkv[2 * C:3 * C][sl]
        in_maps.append({
            "xT": np.ascontiguousarray(x[b].T).astype(NP_MM_DT),
            "wqkv": np.ascontiguousarray(
                np.concatenate([wq, wk, wv], axis=1)).astype(NP_MM_DT),
            "wproj": np.ascontiguousarray(W_proj[sl, :]).astype(NP_MM_DT),
            "bqk": np.ascontiguousarray(
                np.stack(list(bq.reshape(4, 128)) + list(bk.reshape(4, 128)),
                         axis=1)),
            "wvb": np.ascontiguousarray(bv[None, :]).astype(NP_MM_DT),
            "onesr": np.ones((1, 128), NP_MM_DT),
            "m0x2": m0x2,
        })
    return in_maps


def kernel(x, W_qkv, b_qkv, W_proj, b_proj):
    global LAST_EXEC_NS
    x = np.asarray(x, dtype=np.float32)
    W_qkv = np.asarray(W_qkv, dtype=np.float32)
    b_qkv = np.asarray(b_qkv, dtype=np.float32)
    W_proj = np.asarray(W_proj, dtype=np.float32)
    b_proj = np.asarray(b_proj, dtype=np.float32)

    nc = _get_nc()
    in_maps = _shard_inputs(x, W_qkv, b_qkv, W_proj)
    res = run_bass_kernel_spmd(nc, in_maps, core_ids=list(range(NC_CORES)))
    LAST_EXEC_NS = res.exec_time_ns

    out = np.empty((B, T, C), dtype=np.float32)
    for b in range(B):
        out[b] = res.results[2 * b]["out"] + res.results[2 * b + 1]["out"]
        out[b] += b_proj[None, :]
    return out


# revision 17
# speedup vs baseline: 1.2488x; 1.2488x over previous
"""Causal self-attention (B=4, T=2048, C=1024, H=16, D=64) on 8 TRN2 NeuronCores.

Sharding: core c handles batch b = c//2 and head-group hg = c%2 (8 heads each).
Each core computes, for its (b, hg):
  - Q^T, K^T ([512, 2048], head-major transposed) and V ([2048, 512], natural)
    from x[b]^T via f32r matmuls (weights column-sliced per head group)
  - causal attention per head in "S^T layout": S^T[k, q] chunks via bf16
    matmuls, exp on ScalarE (PSUM->SBUF, scale=1/8, no max-subtraction --
    scores are O(few) so exp is safe in f32), causal masking via a
    precomputed step-mask multiply on the diagonal chunks only, and
    PV via matmul with a ones-column appended to V (output row 64 = softmax
    denominator), accumulated in PSUM over k-chunks
  - normalize via reciprocal + partition-broadcast + multiply -> y^T
  - output projection y^T @ W_proj[hg rows] -> partial out [2048, 1024]
Host sums the two head-group partials per batch and adds b_proj.
"""

import os

import numpy as np
import ml_dtypes

import concourse.bass as bass
import concourse.tile as tile
from concourse import bacc, mybir
from concourse.bass_utils import run_bass_kernel_spmd

F32 = mybir.dt.float32
F32R = mybir.dt.float32r
BF16 = mybir.dt.bfloat16
# dtype for the projection (QKV / out-proj) matmul operands: bf16 is 2x faster
# on the PE xbus than f32r (2B/cycle/partition stream width)
MM_DT = BF16 if os.environ.get("KMMDT", "bf16") == "bf16" else F32R
NP_MM_DT = ml_dtypes.bfloat16 if os.environ.get("KMMDT", "bf16") == "bf16" else np.float32
EXP = mybir.ActivationFunctionType.Exp

B, T, C = 4, 2048, 1024
H, D = 16, 64
HPC = 8          # heads per core
CC = C // 128    # 8 contraction chunks
G = T // 512     # 4 q-blocks of 512
NC_CORES = 8

LAST_EXEC_NS = None

_NC_CACHE = None





def _build_nc():
    stage = os.environ.get("KSTAGE", "all")  # debug gate: qkv | attn | nobcast | all
    nc = bacc.Bacc("TRN2", target_bir_lowering=False, debug=False)

    xT_d = nc.dram_tensor("xT", [C, T], MM_DT, kind="ExternalInput")
    wqkv_d = nc.dram_tensor("wqkv", [C, 1536], MM_DT, kind="ExternalInput")
    wproj_d = nc.dram_tensor("wproj", [512, C], MM_DT, kind="ExternalInput")
    bqk_d = nc.dram_tensor("bqk", [128, 8], F32, kind="ExternalInput")
    wvb_d = nc.dram_tensor("wvb", [1, 512], MM_DT, kind="ExternalInput")
    onesr_d = nc.dram_tensor("onesr", [1, 128], MM_DT, kind="ExternalInput")
    m0_d = nc.dram_tensor("m0x2", [128, 256], BF16, kind="ExternalInput")
    out_d = nc.dram_tensor("out", [T, C], F32, kind="ExternalOutput")

    with tile.TileContext(nc) as tc:
        with (
            tc.tile_pool(name="consts", bufs=1) as cp,
            tc.tile_pool(name="kv", bufs=1) as kvp,
            tc.tile_pool(name="stream", bufs=1) as sp,
            tc.tile_pool(name="ps", bufs=1, space="PSUM") as ps,
        ):
            # ---- constants / weights ----
            wqkv = []
            for c in range(CC):
                w = sp.tile([128, 1536], MM_DT, tag=f"wq{c}", name=f"wqkv{c}")
                nc.sync.dma_start(w[:], wqkv_d[c * 128:(c + 1) * 128, :])
                wqkv.append(w)
            wproj = []
            for dch in range(4):
                w = cp.tile([128, C], MM_DT, tag=f"wp{dch}", name=f"wproj{dch}")
                nc.sync.dma_start(w[:], wproj_d[dch * 128:(dch + 1) * 128, :])
                wproj.append(w)
            bqk = cp.tile([128, 8], F32, tag="bqk")
            nc.sync.dma_start(bqk[:], bqk_d[:])
            wvb = cp.tile([1, 512], MM_DT, tag="wvb")
            nc.sync.dma_start(wvb[:], wvb_d[:])
            m0 = cp.tile([128, 256], BF16, tag="m0")
            nc.sync.dma_start(m0[:], m0_d[:])
            ones = cp.tile([1, 128], MM_DT, tag="ones")
            nc.sync.dma_start(ones[:], onesr_d[:])

            # ---- persistent K^T and V(+ones) caches ----
            ktz = []
            for h in range(HPC):
                t = kvp.tile([128, T], BF16, tag=f"ktz{h}", name=f"ktz{h}")
                nc.gpsimd.memset(t[:], 0.0)
                ktz.append(t)
            vo = []
            for t16 in range(16):
                v = kvp.tile([128, HPC * 65 + 64], BF16, tag=f"vo{t16}", name=f"vo{t16}")
                nc.gpsimd.memset(v[:], 1.0)
                vo.append(v)

            for g in range(G):
                # ================= QKV projection for t-block g =================
                xts = []
                for c in range(CC):
                    xt = sp.tile([128, 512], MM_DT, tag="xt", bufs=10, name=f"xt{g}_{c}")
                    nc.sync.dma_start(
                        xt[:], xT_d[c * 128:(c + 1) * 128, g * 512:(g + 1) * 512]
                    )
                    xts.append(xt)

                # Q^T for this q-block: 4 partition-tiles (2 heads each)
                qts = []
                for hp in range(4):
                    p = ps.tile([128, 512], F32, tag="qk", bufs=2, name=f"psq{g}_{hp}")
                    for c in range(CC):
                        nc.tensor.matmul(
                            p[:],
                            wqkv[c][:, hp * 128:(hp + 1) * 128],
                            xts[c][:],
                            start=(c == 0),
                            stop=(c == CC - 1),
                        )
                    q = sp.tile([128, 512], BF16, tag=f"qt{hp}", bufs=2,
                                name=f"qt{g}_{hp}")
                    nc.vector.tensor_scalar_add(q[:], p[:], bqk[:, hp:hp + 1])
                    qts.append(q)

                # K^T block g -> persistent cache
                for hp in range(4):
                    p = ps.tile([128, 512], F32, tag="qk", bufs=2, name=f"psk{g}_{hp}")
                    for c in range(CC):
                        nc.tensor.matmul(
                            p[:],
                            wqkv[c][:, 512 + hp * 128:512 + (hp + 1) * 128],
                            xts[c][:],
                            start=(c == 0),
                            stop=(c == CC - 1),
                        )
                    gs = slice(g * 512, (g + 1) * 512)
                    nc.vector.tensor_scalar_add(
                        ktz[2 * hp][0:64, gs], p[0:64, :],
                        bqk[0:64, 4 + hp:5 + hp]
                    )
                    nc.vector.tensor_scalar_add(
                        ktz[2 * hp + 1][64:128, gs], p[64:128, :],
                        bqk[64:128, 4 + hp:5 + hp]
                    )

                # V natural for 4 t-chunks of block g -> Vones cache
                for t4 in range(4):
                    t16 = 4 * g + t4
                    p = ps.tile([128, 512], F32, tag="qk", bufs=2, name=f"psv{t16}")
                    for c in range(CC):
                        nc.tensor.matmul(
                            p[:],
                            xts[c][:, t4 * 128:(t4 + 1) * 128],
                            wqkv[c][:, 1024:1536],
                            start=(c == 0),
                            stop=False,
                        )
                    nc.tensor.matmul(
                        p[:], ones[:], wvb[:], start=False, stop=True
                    )
                    vslice = vo[t16][:, 0:HPC * 65].rearrange("p (h c) -> p h c", h=HPC)[:, :, 0:64]
                    psrc = p[:].rearrange("p (h c) -> p h c", h=HPC)
                    nc.vector.tensor_copy(vslice, psrc)

                # ================= attention for q-block g =================
                if stage == "qkv":
                    continue
                # chunk descriptors: (kc, q-window offset, width, needs_mask)
                chunks = [(kc, 0, 512, False) for kc in range(4 * g)]
                chunks += [(4 * g + j, 128 * j, 512 - 128 * j, True)
                           for j in range(4)]
                nch = len(chunks)
                for hp in range(4):
                    pvs = []
                    for hs in range(2):
                        pv = ps.tile([128, 512], F32, tag="pv", bufs=2,
                                     name=f"pv{g}_{hp}_{hs}")
                        pvs.append(pv)
                    es = {}

                    def emit_pv(idx):
                        kc, qoff, w, _ = chunks[idx]
                        for hs in range(2):
                            h = 2 * hp + hs
                            nc.tensor.matmul(
                                pvs[hs][:, qoff:qoff + w],
                                vo[kc][:, h * 65:h * 65 + 128],
                                es[(idx, hs)][:, 0:w],
                                start=(kc == 0), stop=(kc == 4 * g + 3),
                            )

                    for idx, (kc, qoff, w, masked) in enumerate(chunks):
                        sgs = []
                        for hs in range(2):
                            sg = ps.tile([128, 512], F32, tag="sg", bufs=4,
                                         name=f"s{g}_{hp}_{hs}_{kc}")
                            nc.tensor.matmul(
                                sg[:, 0:w],
                                ktz[2 * hp + hs][:, kc * 128:(kc + 1) * 128],
                                qts[hp][:, qoff:qoff + w],
                                start=True, stop=True,
                            )
                            sgs.append(sg)
                        for hs in range(2):
                            e = sp.tile([128, 512], BF16, tag="e", bufs=8,
                                        name=f"e{g}_{hp}_{hs}_{kc}")
                            nc.scalar.activation(e[:, 0:w], sgs[hs][:, 0:w],
                                                 EXP, scale=0.125)
                            if masked:
                                nc.vector.tensor_mul(
                                    e[:, 0:128], e[:, 0:128], m0[:, 0:128]
                                )
                            es[(idx, hs)] = e
                        if idx >= 2:
                            emit_pv(idx - 2)
                    for idx in range(max(0, nch - 2), nch):
                        emit_pv(idx)

                    # normalize both heads of the pair -> y tile rows
                    if hp == 0:
                        yts = []
                    yt = sp.tile([128, 512], MM_DT, tag=f"yt{hp}", bufs=2,
                                 name=f"yt{g}_{hp}")
                    for hs in range(2):
                        pv = pvs[hs]
                        if stage in ("attn", "nobcast"):
                            nc.vector.tensor_copy(
                                yt[64 * hs:64 * hs + 64, :], pv[0:64, :]
                            )
                            continue
                        # evacuate the PV accumulator promptly (frees the bank)
                        yU = sp.tile([64, 512], F32, tag="yU", bufs=3,
                                     name=f"yU{g}_{hp}_{hs}")
                        nc.vector.tensor_copy(yU[:], pv[0:64, :])
                        sc = sp.tile([1, 512], F32, tag="sc", bufs=2,
                                     name=f"sc{g}_{hp}_{hs}")
                        nc.scalar.copy(sc[:], pv[64:65, :])
                        r = sp.tile([1, 512], F32, tag="r", bufs=2,
                                    name=f"r{g}_{hp}_{hs}")
                        nc.vector.reciprocal_approx_fast(out=r[:], in_=sc[:])
                        rb = sp.tile([64, 512], F32, tag="rb", bufs=2,
                                     name=f"rb{g}_{hp}_{hs}")
                        nc.gpsimd.partition_broadcast(rb[:], r[:])
                        nc.vector.tensor_mul(
                            yt[64 * hs:64 * hs + 64, :], yU[:], rb[:]
                        )
                    yts.append(yt)

                # ================= output projection for t-block g =================
                if stage == "attn":
                    continue
                for t4 in range(4):
                    for c2 in range(2):
                        p = ps.tile([128, 512], F32, tag="qk", bufs=2,
                                    name=f"pso{g}_{t4}_{c2}")
                        for dch in range(4):
                            nc.tensor.matmul(
                                p[:],
                                yts[dch][:, t4 * 128:(t4 + 1) * 128],
                                wproj[dch][:, c2 * 512:(c2 + 1) * 512],
                                start=(dch == 0),
                                stop=(dch == 3),
                            )
                        o = sp.tile([128, 512], F32, tag="o", bufs=4,
                                    name=f"o{g}_{t4}_{c2}")
                        nc.vector.tensor_copy(o[:], p[:])
                        nc.sync.dma_start(
                            out_d[g * 512 + t4 * 128: g * 512 + (t4 + 1) * 128,
                                  c2 * 512:(c2 + 1) * 512],
                            o[:],
                        )

    nc.compile()
    return nc


def _get_nc():
    global _NC_CACHE
    if _NC_CACHE is None:
        _NC_CACHE = _build_nc()
    return _NC_CACHE


def _shard_inputs(x, W_qkv, b_qkv, W_proj):
    """Build the per-core input maps."""
    k0 = np.arange(128)
    m0 = (k0[:, None] <= np.arange(128)[None, :]).astype(ml_dtypes.bfloat16)
    m0x2 = np.concatenate([m0, m0], axis=1)

    in_maps = []
    for core in range(NC_CORES):
        b, hg = core // 2, core % 2
        sl = slice(hg * 512, (hg + 1) * 512)
        wq = W_qkv[:, 0 * C:1 * C][:, sl]
        wk = W_qkv[:, 1 * C:2 * C][:, sl]
        wv = W_qkv[:, 2 * C:3 * C][:, sl]
        bq = b_qkv[0 * C:1 * C][sl]
        bk = b_qkv[1 * C:2 * C][sl]
        bv = b_qkv[2 * C:3 * C][sl]
        in_maps.append({
            "xT": np.ascontiguousarray(x[b].T).astype(NP_MM_DT),
            "wqkv": np.ascontiguousarray(
                np.concatenate([wq, wk, wv], axis=1)).astype(NP_MM_DT),
            "wproj": np.ascontiguousarray(W_proj[sl, :]).astype(NP_MM_DT),
            "bqk": np.ascontiguousarray(
                np.stack(list(bq.reshape(4, 128)) + list(bk.reshape(4, 128)),
                         axis=1)),
            "wvb": np.ascontiguousarray(bv[None, :]).astype(NP_MM_DT),
            "onesr": np.ones((1, 128), NP_MM_DT),
            "m0x2": m0x2,
        })
    return in_maps


def kernel(x, W_qkv, b_qkv, W_proj, b_proj):
    global LAST_EXEC_NS
    x = np.asarray(x, dtype=np.float32)
    W_qkv = np.asarray(W_qkv, dtype=np.float32)
    b_qkv = np.asarray(b_qkv, dtype=np.float32)
    W_proj = np.asarray(W_proj, dtype=np.float32)
    b_proj = np.asarray(b_proj, dtype=np.float32)

    nc = _get_nc()
    in_maps = _shard_inputs(x, W_qkv, b_qkv, W_proj)
    res = run_bass_kernel_spmd(nc, in_maps, core_ids=list(range(NC_CORES)))
    LAST_EXEC_NS = res.exec_time_ns

    out = np.empty((B, T, C), dtype=np.float32)
    for b in range(B):
        out[b] = res.results[2 * b]["out"] + res.results[2 * b + 1]["out"]
        out[b] += b_proj[None, :]
    return out


# revision 19
# speedup vs baseline: 1.3184x; 1.0558x over previous
"""Causal self-attention (B=4, T=2048, C=1024, H=16, D=64) on 8 TRN2 NeuronCores.

Sharding: core c handles batch b = c//2 and head-group hg = c%2 (8 heads each).
Each core computes, for its (b, hg):
  - Q^T, K^T ([512, 2048], head-major transposed) and V ([2048, 512], natural)
    from x[b]^T via f32r matmuls (weights column-sliced per head group)
  - causal attention per head in "S^T layout": S^T[k, q] chunks via bf16
    matmuls, exp on ScalarE (PSUM->SBUF, scale=1/8, no max-subtraction --
    scores are O(few) so exp is safe in f32), causal masking via a
    precomputed step-mask multiply on the diagonal chunks only, and
    PV via matmul with a ones-column appended to V (output row 64 = softmax
    denominator), accumulated in PSUM over k-chunks
  - normalize via reciprocal + partition-broadcast + multiply -> y^T
  - output projection y^T @ W_proj[hg rows] -> partial out [2048, 1024]
Host sums the two head-group partials per batch and adds b_proj.
"""

import os

import numpy as np
import ml_dtypes

import concourse.bass as bass
import concourse.tile as tile
from concourse import bacc, mybir
from concourse.bass_utils import run_bass_kernel_spmd

F32 = mybir.dt.float32
F32R = mybir.dt.float32r
BF16 = mybir.dt.bfloat16
# dtype for the projection (QKV / out-proj) matmul operands: bf16 is 2x faster
# on the PE xbus than f32r (2B/cycle/partition stream width)
MM_DT = BF16 if os.environ.get("KMMDT", "bf16") == "bf16" else F32R
NP_MM_DT = ml_dtypes.bfloat16 if os.environ.get("KMMDT", "bf16") == "bf16" else np.float32
EXP = mybir.ActivationFunctionType.Exp

B, T, C = 4, 2048, 1024
H, D = 16, 64
HPC = 8          # heads per core
CC = C // 128    # 8 contraction chunks
G = T // 512     # 4 q-blocks of 512
NC_CORES = 8

LAST_EXEC_NS = None

_NC_CACHE = None





def _build_nc():
    stage = os.environ.get("KSTAGE", "all")  # debug gate: qkv | attn | nobcast | all
    nc = bacc.Bacc("TRN2", target_bir_lowering=False, debug=False)

    xT_d = nc.dram_tensor("xT", [C, T], MM_DT, kind="ExternalInput")
    wqkv_d = nc.dram_tensor("wqkv", [C, 1536], MM_DT, kind="ExternalInput")
    wproj_d = nc.dram_tensor("wproj", [512, C], MM_DT, kind="ExternalInput")
    bqk_d = nc.dram_tensor("bqk", [128, 8], F32, kind="ExternalInput")
    wvb_d = nc.dram_tensor("wvb", [1, 512], MM_DT, kind="ExternalInput")
    onesr_d = nc.dram_tensor("onesr", [1, 128], MM_DT, kind="ExternalInput")
    m0_d = nc.dram_tensor("m0x2", [128, 256], BF16, kind="ExternalInput")
    out_d = nc.dram_tensor("out", [T, C], F32, kind="ExternalOutput")

    with tile.TileContext(nc) as tc:
        with (
            tc.tile_pool(name="consts", bufs=1) as cp,
            tc.tile_pool(name="kv", bufs=1) as kvp,
            tc.tile_pool(name="stream", bufs=1) as sp,
            tc.tile_pool(name="ps", bufs=1, space="PSUM") as ps,
        ):
            # ---- constants / weights ----
            wqkv = []
            for c in range(CC):
                w = sp.tile([128, 1536], MM_DT, tag=f"wq{c}", name=f"wqkv{c}")
                nc.sync.dma_start(w[:], wqkv_d[c * 128:(c + 1) * 128, :])
                wqkv.append(w)
            bqk = cp.tile([128, 8], F32, tag="bqk")
            nc.sync.dma_start(bqk[:], bqk_d[:])
            wvb = cp.tile([1, 512], MM_DT, tag="wvb")
            nc.sync.dma_start(wvb[:], wvb_d[:])
            m0 = cp.tile([128, 256], BF16, tag="m0")
            nc.sync.dma_start(m0[:], m0_d[:])
            ones = cp.tile([1, 128], MM_DT, tag="ones")
            nc.sync.dma_start(ones[:], onesr_d[:])

            # ---- persistent K^T and V(+ones) caches ----
            ktz = []
            for h in range(HPC):
                t = kvp.tile([128, T], BF16, tag=f"ktz{h}", name=f"ktz{h}")
                if h % 2 == 0:
                    nc.vector.memset(t[64:128, :], 0.0)
                else:
                    nc.vector.memset(t[0:64, :], 0.0)
                ktz.append(t)
            vo = []
            for t16 in range(16):
                v = kvp.tile([128, HPC * 65 + 64], BF16, tag=f"vo{t16}", name=f"vo{t16}")
                nc.vector.memset(v[:], 1.0)
                vo.append(v)

            wproj = []
            for g in range(G):
                # ================= QKV projection for t-block g =================
                xts = []
                for c in range(CC):
                    xt = sp.tile([128, 512], MM_DT, tag="xt", bufs=10, name=f"xt{g}_{c}")
                    nc.sync.dma_start(
                        xt[:], xT_d[c * 128:(c + 1) * 128, g * 512:(g + 1) * 512]
                    )
                    xts.append(xt)

                if g == 0:
                    for dch in range(4):
                        w = cp.tile([128, C], MM_DT, tag=f"wp{dch}",
                                    name=f"wproj{dch}")
                        nc.sync.dma_start(
                            w[:], wproj_d[dch * 128:(dch + 1) * 128, :])
                        wproj.append(w)

                # Q^T for this q-block: 4 partition-tiles (2 heads each)
                qts = []
                for hp in range(4):
                    p = ps.tile([128, 512], F32, tag="mm", bufs=6, name=f"psq{g}_{hp}")
                    for c in range(CC):
                        nc.tensor.matmul(
                            p[:],
                            wqkv[c][:, hp * 128:(hp + 1) * 128],
                            xts[c][:],
                            start=(c == 0),
                            stop=(c == CC - 1),
                        )
                    q = sp.tile([128, 512], BF16, tag=f"qt{hp}", bufs=2,
                                name=f"qt{g}_{hp}")
                    nc.vector.tensor_scalar_add(q[:], p[:], bqk[:, hp:hp + 1])
                    qts.append(q)

                # K^T block g -> persistent cache
                for hp in range(4):
                    p = ps.tile([128, 512], F32, tag="mm", bufs=6, name=f"psk{g}_{hp}")
                    for c in range(CC):
                        nc.tensor.matmul(
                            p[:],
                            wqkv[c][:, 512 + hp * 128:512 + (hp + 1) * 128],
                            xts[c][:],
                            start=(c == 0),
                            stop=(c == CC - 1),
                        )
                    gs = slice(g * 512, (g + 1) * 512)
                    nc.vector.tensor_scalar_add(
                        ktz[2 * hp][0:64, gs], p[0:64, :],
                        bqk[0:64, 4 + hp:5 + hp]
                    )
                    nc.vector.tensor_scalar_add(
                        ktz[2 * hp + 1][64:128, gs], p[64:128, :],
                        bqk[64:128, 4 + hp:5 + hp]
                    )

                # V natural for 4 t-chunks of block g -> Vones cache
                for t4 in range(4):
                    t16 = 4 * g + t4
                    p = ps.tile([128, 512], F32, tag="mm", bufs=6, name=f"psv{t16}")
                    for c in range(CC):
                        nc.tensor.matmul(
                            p[:],
                            xts[c][:, t4 * 128:(t4 + 1) * 128],
                            wqkv[c][:, 1024:1536],
                            start=(c == 0),
                            stop=False,
                        )
                    nc.tensor.matmul(
                        p[:], ones[:], wvb[:], start=False, stop=True
                    )
                    vslice = vo[t16][:, 0:HPC * 65].rearrange("p (h c) -> p h c", h=HPC)[:, :, 0:64]
                    psrc = p[:].rearrange("p (h c) -> p h c", h=HPC)
                    nc.vector.tensor_copy(vslice, psrc)

                # ================= attention for q-block g =================
                if stage == "qkv":
                    continue
                # chunk descriptors: (kc, q-window offset, width, needs_mask)
                chunks = [(kc, 0, 512, False) for kc in range(4 * g)]
                chunks += [(4 * g + j, 128 * j, 512 - 128 * j, True)
                           for j in range(4)]
                nch = len(chunks)
                for hp in range(4):
                    pvs = []
                    for hs in range(2):
                        pv = ps.tile([128, 512], F32, tag="pv", bufs=2,
                                     name=f"pv{g}_{hp}_{hs}")
                        pvs.append(pv)
                    es = {}

                    def emit_pv(idx):
                        kc, qoff, w, _ = chunks[idx]
                        for hs in range(2):
                            h = 2 * hp + hs
                            nc.tensor.matmul(
                                pvs[hs][:, qoff:qoff + w],
                                vo[kc][:, h * 65:h * 65 + 128],
                                es[(idx, hs)][:, 0:w],
                                start=(kc == 0), stop=(kc == 4 * g + 3),
                            )

                    for idx, (kc, qoff, w, masked) in enumerate(chunks):
                        sgs = []
                        for hs in range(2):
                            sg = ps.tile([128, 512], F32, tag="mm", bufs=6,
                                         name=f"s{g}_{hp}_{hs}_{kc}")
                            nc.tensor.matmul(
                                sg[:, 0:w],
                                ktz[2 * hp + hs][:, kc * 128:(kc + 1) * 128],
                                qts[hp][:, qoff:qoff + w],
                                start=True, stop=True,
                            )
                            sgs.append(sg)
                        for hs in range(2):
                            e = sp.tile([128, 512], BF16, tag="e", bufs=8,
                                        name=f"e{g}_{hp}_{hs}_{kc}")
                            nc.scalar.activation(e[:, 0:w], sgs[hs][:, 0:w],
                                                 EXP, scale=0.125)
                            if masked:
                                nc.vector.tensor_mul(
                                    e[:, 0:128], e[:, 0:128], m0[:, 0:128]
                                )
                            es[(idx, hs)] = e
                        if idx >= 2:
                            emit_pv(idx - 2)
                    for idx in range(max(0, nch - 2), nch):
                        emit_pv(idx)

                    # normalize both heads of the pair -> y tile rows
                    if hp == 0:
                        yts = []
                    yt = sp.tile([128, 512], MM_DT, tag=f"yt{hp}", bufs=2,
                                 name=f"yt{g}_{hp}")
                    for hs in range(2):
                        pv = pvs[hs]
                        if stage in ("attn", "nobcast"):
                            nc.vector.tensor_copy(
                                yt[64 * hs:64 * hs + 64, :], pv[0:64, :]
                            )
                            continue
                        # evacuate the PV accumulator promptly (frees the bank)
                        yU = sp.tile([64, 512], F32, tag="yU", bufs=3,
                                     name=f"yU{g}_{hp}_{hs}")
                        nc.vector.tensor_copy(yU[:], pv[0:64, :])
                        sc = sp.tile([1, 512], F32, tag="sc", bufs=2,
                                     name=f"sc{g}_{hp}_{hs}")
                        nc.scalar.copy(sc[:], pv[64:65, :])
                        r = sp.tile([1, 512], F32, tag="r", bufs=2,
                                    name=f"r{g}_{hp}_{hs}")
                        nc.vector.reciprocal_approx_fast(out=r[:], in_=sc[:])
                        rb = sp.tile([64, 512], F32, tag="rb", bufs=2,
                                     name=f"rb{g}_{hp}_{hs}")
                        nc.gpsimd.partition_broadcast(rb[:], r[:])
                        nc.vector.tensor_mul(
                            yt[64 * hs:64 * hs + 64, :], yU[:], rb[:]
                        )
                    yts.append(yt)

                # ================= output projection for t-block g =================
                if stage == "attn":
                    continue
                for t4 in range(4):
                    for c2 in range(2):
                        p = ps.tile([128, 512], F32, tag="mm", bufs=6,
                                    name=f"pso{g}_{t4}_{c2}")
                        for dch in range(4):
                            nc.tensor.matmul(
                                p[:],
                                yts[dch][:, t4 * 128:(t4 + 1) * 128],
                                wproj[dch][:, c2 * 512:(c2 + 1) * 512],
                                start=(dch == 0),
                                stop=(dch == 3),
                            )
                        o = sp.tile([128, 512], F32, tag="o", bufs=4,
                                    name=f"o{g}_{t4}_{c2}")
                        nc.vector.tensor_copy(o[:], p[:])
                        nc.sync.dma_start(
                            out_d[g * 512 + t4 * 128: g * 512 + (t4 + 1) * 128,
                                  c2 * 512:(c2 + 1) * 512],
                            o[:],
                        )

    nc.compile()
    return nc


def _get_nc():
    global _NC_CACHE
    if _NC_CACHE is None:
        _NC_CACHE = _build_nc()
    return _NC_CACHE


def _shard_inputs(x, W_qkv, b_qkv, W_proj):
    """Build the per-core input maps."""
    k0 = np.arange(128)
    m0 = (k0[:, None] <= np.arange(128)[None, :]).astype(ml_dtypes.bfloat16)
    m0x2 = np.concatenate([m0, m0], axis=1)

    in_maps = []
    for core in range(NC_CORES):
        b, hg = core // 2, core % 2
        sl = slice(hg * 512, (hg + 1) * 512)
        wq = W_qkv[:, 0 * C:1 * C][:, sl]
        wk = W_qkv[:, 1 * C:2 * C][:, sl]
        wv = W_qkv[:, 2 * C:3 * C][:, sl]
        bq = b_qkv[0 * C:1 * C][sl]
        bk = b_qkv[1 * C:2 * C][sl]
        bv = b_qkv[2 * C:3 * C][sl]
        in_maps.append({
            "xT": np.ascontiguousarray(x[b].T).astype(NP_MM_DT),
            "wqkv": np.ascontiguousarray(
                np.concatenate([wq, wk, wv], axis=1)).astype(NP_MM_DT),
            "wproj": np.ascontiguousarray(W_proj[sl, :]).astype(NP_MM_DT),
            "bqk": np.ascontiguousarray(
                np.stack(list(bq.reshape(4, 128)) + list(bk.reshape(4, 128)),
                         axis=1)),
            "wvb": np.ascontiguousarray(bv[None, :]).astype(NP_MM_DT),
            "onesr": np.ones((1, 128), NP_MM_DT),
            "m0x2": m0x2,
        })
    return in_maps


def kernel(x, W_qkv, b_qkv, W_proj, b_proj):
    global LAST_EXEC_NS
    x = np.asarray(x, dtype=np.float32)
    W_qkv = np.asarray(W_qkv, dtype=np.float32)
    b_qkv = np.asarray(b_qkv, dtype=np.float32)
    W_proj = np.asarray(W_proj, dtype=np.float32)
    b_proj = np.asarray(b_proj, dtype=np.float32)

    nc = _get_nc()
    in_maps = _shard_inputs(x, W_qkv, b_qkv, W_proj)
    res = run_bass_kernel_spmd(nc, in_maps, core_ids=list(range(NC_CORES)))
    LAST_EXEC_NS = res.exec_time_ns

    out = np.empty((B, T, C), dtype=np.float32)
    for b in range(B):
        out[b] = res.results[2 * b]["out"] + res.results[2 * b + 1]["out"]
        out[b] += b_proj[None, :]
    return out
